# revision 38
# baseline (speedup 1.0000x reference)
"""Mixtral sparse-MoE block on 8 Trainium2 NeuronCores (expert parallel).

Strategy: expert-parallel with token dispatch (capacity-based routing).
The router (softmax + top-2 + renormalize) is part of deciding the shard:
it is evaluated on the host in float64 (bit-stable ordering; verified to
reproduce the jax float32 reference selection exactly), and each core
receives only the tokens routed to its expert, gathered into a fixed
[H, C] capacity buffer (C=1088 covers the max per-expert load ~1086 with
margin; small overflows are computed on the host in f32 and large ones
fall back to a second device pass, so any routing stays correct).

Each core then runs its expert's up/gate/down matmuls on its C tokens in
fp16 (same PE cycles/row as fp32r on TRN2, half the DMA/SBUF), applies the
per-token combine weight on-chip, and writes its [H, C] contribution.
The host scatter-adds the two expert contributions per token (top-2), so
no device collective is needed at all.

Compared to the dense formulation (every expert computes all T tokens,
then psum), this does 1088/4096 = 27% of the PE work per core.

Layouts: feature-major on-chip ("xg" = gathered x transposed) so all three
expert matmuls keep weights as the stationary operand and tokens as the
moving free dim, with zero on-chip transposes. Weights are pre-tiled on
the host to [128, n_tiles, k, 128] so every weight DMA moves 2KB-contiguous
per-partition lines (full DMA bus rate).
"""

import numpy as np

import concourse.mybir as mybir
import concourse.tile as tile
from concourse import bacc
from concourse.bass_utils import run_bass_kernel_spmd

# Problem shape (hardcoded per contract).
B, S, H, F, E = 2, 2048, 1024, 2048, 8
T = B * S                    # 4096 tokens
N_CORES = 8
HC = H // 128                # 8 h-chunks
FC = F // 128                # 16 f-chunks
C = 1088                     # per-expert token capacity (max load ~1086)
CHUNKS = [(0, 512), (512, 512), (1024, 64)]    # token chunks of C
OVERFLOW_MAX = 512           # beyond this, fall back to device multi-pass
NT = len(CHUNKS)

f32 = mybir.dt.float32
f16 = mybir.dt.float16


def build():
    nc = bacc.Bacc("TRN2", target_bir_lowering=False, debug=False,
                   num_devices=N_CORES)

    xg = nc.dram_tensor("xg", [H, C], f16, kind="ExternalInput").ap()
    # Weights arrive pre-tiled: [p, tile, k, 128] so a single tile's DMA is
    # contiguous per partition (2KB lines -> full DMA rate).
    wu = nc.dram_tensor("wu", [128, FC, HC, 128], f16,
                        kind="ExternalInput").ap()
    wg = nc.dram_tensor("wg", [128, FC, HC, 128], f16,
                        kind="ExternalInput").ap()
    wd = nc.dram_tensor("wd", [128, HC, FC, 128], f16,
                        kind="ExternalInput").ap()
    cw = nc.dram_tensor("cw", [128, C], f32, kind="ExternalInput").ap()
    yp = nc.dram_tensor("yp", [H, C], f32, kind="ExternalOutput").ap()

    xg_v = xg.rearrange("(hc p) t -> p hc t", p=128)     # [128, 8, C]
    yp_v = yp.rearrange("(hc p) t -> p hc t", p=128)     # [128, 8, C]

    with tile.TileContext(nc) as tc:
        with (
            tc.tile_pool(name="const", bufs=1) as cpool,
            tc.tile_pool(name="w", bufs=2) as wpool,
            tc.tile_pool(name="inner", bufs=1) as ipool,
            tc.tile_pool(name="work", bufs=3) as spool,
            tc.tile_pool(name="psum", bufs=2, space="PSUM") as psum,
        ):
            # PE clock warm-up: the tensor engine p-state ramps over ~3us of
            # continuous execution. Burn that ramp on dummy matmuls (no DMA
            # deps, SBUF tile memset by Pool) while the first input DMAs are
            # in flight, so real matmuls run at full clock from the start.
            warm = cpool.tile([128, 512], f16, tag="warm")
            nc.vector.memset(warm[:], 0.0)
            for _ in range(6):
                wps = psum.tile([128, 512], f32, tag="up", bufs=4)
                nc.tensor.matmul(wps[:], warm[:, :128], warm[:],
                                 start=True, stop=True)

            # All up/gate weights stay resident in SBUF (8 MB), loaded once.
            wu_sb = [cpool.tile([128, HC, 128], f16, tag=f"wu{fc}",
                                name=f"wu{fc}") for fc in range(FC)]
            wg_sb = [cpool.tile([128, HC, 128], f16, tag=f"wg{fc}",
                                name=f"wg{fc}") for fc in range(FC)]
            xq = [cpool.tile([128, HC, n], f16, tag=f"xg{tn}",
                             name=f"xg{tn}")
                  for tn, (st, n) in enumerate(CHUNKS)]

            # DMA issue order is latency-critical at the start: fc0 weights,
            # then chunk-0 tokens in 2-hc pieces (so the first up-chain
            # starts after ~0.5 MB instead of the whole input), then the
            # next couple of weight tiles, then everything else. cw is only
            # needed in phase C so it loads last.
            def load_w(fc):
                nc.sync.dma_start(wu_sb[fc][:], wu[:, fc, :, :])
                nc.sync.dma_start(wg_sb[fc][:], wg[:, fc, :, :])

            # First up-chain's deps land finest-grained and first: half of
            # wu0, first xg piece, rest of wu0, next xg piece, wg0, ...
            # Alternate the first loads between sync (shared HWDGE) and
            # gpsimd (its own SWDGE) so descriptor generation pipelines run
            # in parallel and the serial DMA bus never starves on gens.
            nc.sync.dma_start(wu_sb[0][:, 0:4, :], wu[:, 0, 0:4, :])
            nc.gpsimd.dma_start(xq[0][:, 0:2, :], xg_v[:, 0:2, 0:512])
            nc.sync.dma_start(xq[0][:, 2:4, :], xg_v[:, 2:4, 0:512])
            nc.gpsimd.dma_start(wu_sb[0][:, 4:8, :], wu[:, 0, 4:8, :])
            nc.sync.dma_start(wg_sb[0][:], wg[:, 0, :, :])
            nc.gpsimd.dma_start(xq[0][:, 4:6, :], xg_v[:, 4:6, 0:512])
            nc.sync.dma_start(xq[0][:, 6:8, :], xg_v[:, 6:8, 0:512])
            load_w(1)
            nc.sync.dma_start(xq[2][:], xg_v[:, :, 1024:C])
            for fc in range(2, 6):
                load_w(fc)
            for hp in range(0, HC, 2):
                nc.sync.dma_start(xq[1][:, hp:hp + 2, :],
                                  xg_v[:, hp:hp + 2, 512:1024])
            for fc in range(6, 9):
                load_w(fc)
            cw_sb = cpool.tile([128, C], f32)
            nc.sync.dma_start(cw_sb[:], cw[:])
            for fc in range(9, FC):
                load_w(fc)

            # ---- Phase A: up/gate matmuls + silu -> inner (fp16) ----
            # tn0 pass first (only needs xg chunk 0, so PE reaches steady
            # state as soon as fc0's weights and the first xg piece land),
            # with each short tn2 tile interleaved between the long
            # tn0 chains (64-wide): its Act/DVE consumers then have a 3.4us chain of
            # slack, instead of stalling the PE sequencer back-to-back.
            inner = [[None] * NT for _ in range(FC)]

            def ffn_tile(tn, fc):
                st, n = CHUNKS[tn]
                up_ps = psum.tile([128, 512], f32, tag="up", bufs=4,
                                  name="up_ps")
                for hc in range(HC):
                    nc.tensor.matmul(up_ps[:, :n], wu_sb[fc][:, hc, :],
                                     xq[tn][:, hc, :],
                                     start=(hc == 0), stop=(hc == HC - 1))
                # "y" tag shared with phase C (never live together): 4+4
                # buffers across both tags hides chain-start sem waits.
                gate_ps = psum.tile([128, 512], f32, tag="y", bufs=4,
                                    name="gate_ps")
                for hc in range(HC):
                    nc.tensor.matmul(gate_ps[:, :n], wg_sb[fc][:, hc, :],
                                     xq[tn][:, hc, :],
                                     start=(hc == 0), stop=(hc == HC - 1))
                sg_sb = spool.tile([128, 512], f32, tag="sg", name="sg_sb")
                nc.scalar.activation(sg_sb[:, :n], up_ps[:, :n],
                                     mybir.ActivationFunctionType.Sigmoid)
                silu_sb = spool.tile([128, 512], f32, tag="silu",
                                     name="silu_sb")
                nc.vector.tensor_mul(silu_sb[:, :n], sg_sb[:, :n],
                                     up_ps[:, :n])
                it = ipool.tile([128, n], f16, tag=f"i{fc}_{tn}",
                                name=f"i{fc}_{tn}")
                nc.vector.tensor_mul(it[:], silu_sb[:, :n], gate_ps[:, :n])
                inner[fc][tn] = it

            for fc in range(FC):
                ffn_tile(0, fc)
                if fc >= 1:
                    ffn_tile(2, fc - 1)
            ffn_tile(2, FC - 1)
            for fc in range(FC):
                ffn_tile(1, fc)

            # ---- Phase C: down matmul + combine scale ----
            for hc in range(HC):
                wd_t = wpool.tile([128, FC, 128], f16, tag="wd")
                nc.sync.dma_start(wd_t[:], wd[:, hc, :, :])
                for tn, (st, n) in enumerate(CHUNKS):
                    if hc == HC - 1 and tn == 1:
                        # Second-to-last chunk: run it as two 256-wide
                        # half-chains so each half's mul+store flows while
                        # the next half computes -- its DMA-completion
                        # semaphore then fires before the final chunk's,
                        # taking this store off the drain's critical path.
                        for h0, h1, eng in ((0, 256, nc.gpsimd),
                                            (256, n, nc.sync)):
                            yh_ps = psum.tile([128, 512], f32, tag="y",
                                              bufs=4, name="yh_ps")
                            for fcj in range(FC):
                                nc.tensor.matmul(
                                    yh_ps[:, :h1 - h0], wd_t[:, fcj, :],
                                    inner[fcj][tn][:, h0:h1],
                                    start=(fcj == 0), stop=(fcj == FC - 1))
                            yh_sb = spool.tile([128, 512], f32, tag="ysb",
                                               name="yh_sb")
                            nc.vector.tensor_mul(yh_sb[:, :h1 - h0],
                                                 yh_ps[:, :h1 - h0],
                                                 cw_sb[:, st + h0:st + h1])
                            eng.dma_start(yp_v[:, hc, st + h0:st + h1],
                                          yh_sb[:, :h1 - h0])
                        continue
                    y_ps = psum.tile([128, 512], f32, tag="y", bufs=4)
                    for fcj in range(FC):
                        nc.tensor.matmul(y_ps[:, :n], wd_t[:, fcj, :],
                                         inner[fcj][tn][:],
                                         start=(fcj == 0),
                                         stop=(fcj == FC - 1))
                    y_sb = spool.tile([128, 512], f32, tag="ysb")
                    nc.vector.tensor_mul(y_sb[:, :n], y_ps[:, :n],
                                         cw_sb[:, st:st + n])
                    # Alternate store queues (Pool / Act) so consecutive
                    # stores don't serialize behind one DGE; the very last
                    # store gets the Act queue to itself so the tail is
                    # just mul -> issue -> 48KB transfer -> drain.
                    if hc == HC - 1:
                        eng = nc.scalar if tn == NT - 1 else nc.gpsimd
                    else:
                        eng = (nc.gpsimd if (hc * NT + tn) % 2 == 0
                               else nc.scalar)
                    eng.dma_start(yp_v[:, hc, st:st + n], y_sb[:, :n])

    nc.compile()
    return nc


_CACHED = None


def _get_program():
    global _CACHED
    if _CACHED is None:
        _CACHED = build()
    return _CACHED


def _route(x, gw):
    """Host router: float64 logits give the true prob ordering (softmax is
    monotonic, so top-2 of probs == top-2 of logits); verified to match the
    jax f32 reference selection exactly on the reference inputs."""
    logits = x.astype(np.float64) @ gw.astype(np.float64)
    m = logits.max(axis=-1, keepdims=True)
    e = np.exp(logits - m)
    probs = e / e.sum(axis=-1, keepdims=True)
    ar = np.arange(T)
    i1 = probs.argmax(axis=-1)
    p2 = probs.copy()
    p2[ar, i1] = -np.inf
    i2 = p2.argmax(axis=-1)
    w1 = probs[ar, i1]
    w2 = probs[ar, i2]
    s = w1 + w2
    return i1, i2, (w1 / s).astype(np.float32), (w2 / s).astype(np.float32)


def _tile_ug(w):
    """[H, F] -> [128, FC, HC, 128] fp16 (p, f-tile, h-row, f)."""
    return np.ascontiguousarray(
        np.asarray(w, np.float16).reshape(HC, 128, FC, 128)
        .transpose(1, 2, 0, 3))


def _tile_d(w):
    """[F, H] -> [128, HC, FC, 128] fp16 (p, h-tile, f-row, h)."""
    return np.ascontiguousarray(
        np.asarray(w, np.float16).reshape(FC, 128, HC, 128)
        .transpose(1, 2, 0, 3))


def kernel(hidden_states, gate_w, w_up, w_gate, w_down):
    nc = _get_program()
    x = np.asarray(hidden_states, np.float32).reshape(T, H)
    gw = np.asarray(gate_w, np.float32)
    i1, i2, w1, w2 = _route(x, gw)

    # Per-expert token index lists + combine weights.
    sel = np.concatenate([i1, i2])              # [2T]
    wgt = np.concatenate([w1, w2])              # [2T]
    tok = np.concatenate([np.arange(T), np.arange(T)])
    order = np.argsort(sel, kind="stable")
    sel_s, wgt_s, tok_s = sel[order], wgt[order], tok[order]
    bounds = np.searchsorted(sel_s, np.arange(E + 1))
    idx_e = [tok_s[bounds[e]:bounds[e + 1]] for e in range(E)]
    wgt_e = [wgt_s[bounds[e]:bounds[e + 1]] for e in range(E)]

    wu16 = [_tile_ug(w_up[e]) for e in range(E)]
    wg16 = [_tile_ug(w_gate[e]) for e in range(E)]
    wd16 = [_tile_d(w_down[e]) for e in range(E)]

    y = np.zeros((T, H), np.float32)
    max_n = max(len(ix) for ix in idx_e)
    overflow = sum(max(0, len(ix) - C) for ix in idx_e)
    host_tail = 0 < overflow <= OVERFLOW_MAX
    # 1 pass normally; a handful of overflow tokens (count drift in a
    # different environment) go to the host in f32, anything larger gets
    # more device passes.
    n_pass = 1 if host_tail else max(1, -(-max_n // C))
    for p in range(n_pass):
        in_maps = []
        ns = []
        for e in range(E):
            ix = idx_e[e][p * C:(p + 1) * C]
            w = wgt_e[e][p * C:(p + 1) * C]
            n = len(ix)
            ns.append(n)
            xg = np.zeros((H, C), np.float16)
            if n:
                xg[:, :n] = x[ix].T
            cwb = np.zeros((128, C), np.float32)
            if n:
                cwb[:, :n] = w[None, :]
            in_maps.append({
                "xg": xg, "wu": wu16[e], "wg": wg16[e], "wd": wd16[e],
                "cw": cwb,
            })
        res = run_bass_kernel_spmd(nc, in_maps, list(range(N_CORES)))
        for e in range(E):
            n = ns[e]
            if n:
                ix = idx_e[e][p * C:(p + 1) * C]
                y[ix] += np.asarray(res.results[e]["yp"][:, :n].T, np.float32)

    if host_tail:
        def silu(v):
            return v / (1.0 + np.exp(-v))
        for e in range(E):
            ix = idx_e[e][C:]
            if not len(ix):
                continue
            w = wgt_e[e][C:]
            xo = x[ix]                                      # [m, H] f32
            wu_f = np.asarray(w_up[e], np.float32)
            wg_f = np.asarray(w_gate[e], np.float32)
            wd_f = np.asarray(w_down[e], np.float32)
            inner_o = silu(xo @ wu_f) * (xo @ wg_f)
            y[ix] += (inner_o @ wd_f) * w[:, None]

    return y.reshape(B, S, H)


# revision 39
# speedup vs baseline: 1.0070x; 1.0070x over previous
"""Mixtral sparse-MoE block on 8 Trainium2 NeuronCores (expert parallel).

Strategy: expert-parallel with token dispatch (capacity-based routing).
The router (softmax + top-2 + renormalize) is part of deciding the shard:
it is evaluated on the host in float64 (bit-stable ordering; verified to
reproduce the jax float32 reference selection exactly), and each core
receives only the tokens routed to its expert, gathered into a fixed
[H, C] capacity buffer (C=1088 covers the max per-expert load ~1086 with
margin; small overflows are computed on the host in f32 and large ones
fall back to a second device pass, so any routing stays correct).

Each core then runs its expert's up/gate/down matmuls on its C tokens in
fp16 (same PE cycles/row as fp32r on TRN2, half the DMA/SBUF), applies the
per-token combine weight on-chip, and writes its [H, C] contribution.
The host scatter-adds the two expert contributions per token (top-2), so
no device collective is needed at all.

Compared to the dense formulation (every expert computes all T tokens,
then psum), this does 1088/4096 = 27% of the PE work per core.

Layouts: feature-major on-chip ("xg" = gathered x transposed) so all three
expert matmuls keep weights as the stationary operand and tokens as the
moving free dim, with zero on-chip transposes. Weights are pre-tiled on
the host to [128, n_tiles, k, 128] so every weight DMA moves 2KB-contiguous
per-partition lines (full DMA bus rate).
"""

import numpy as np

import concourse.mybir as mybir
import concourse.tile as tile
from concourse import bacc
from concourse.bass_utils import run_bass_kernel_spmd

# Problem shape (hardcoded per contract).
B, S, H, F, E = 2, 2048, 1024, 2048, 8
T = B * S                    # 4096 tokens
N_CORES = 8
HC = H // 128                # 8 h-chunks
FC = F // 128                # 16 f-chunks
C = 1088                     # per-expert token capacity (max load ~1086)
CHUNKS = [(0, 512), (512, 512), (1024, 64)]    # token chunks of C
OVERFLOW_MAX = 512           # beyond this, fall back to device multi-pass
NT = len(CHUNKS)

f32 = mybir.dt.float32
f16 = mybir.dt.float16


def build():
    nc = bacc.Bacc("TRN2", target_bir_lowering=False, debug=False,
                   num_devices=N_CORES)

    xg = nc.dram_tensor("xg", [H, C], f16, kind="ExternalInput").ap()
    # Weights arrive pre-tiled: [p, tile, k, 128] so a single tile's DMA is
    # contiguous per partition (2KB lines -> full DMA rate).
    wu = nc.dram_tensor("wu", [128, FC, HC, 128], f16,
                        kind="ExternalInput").ap()
    wg = nc.dram_tensor("wg", [128, FC, HC, 128], f16,
                        kind="ExternalInput").ap()
    wd = nc.dram_tensor("wd", [128, HC, FC, 128], f16,
                        kind="ExternalInput").ap()
    cw = nc.dram_tensor("cw", [128, C], f32, kind="ExternalInput").ap()
    yp = nc.dram_tensor("yp", [H, C], f32, kind="ExternalOutput").ap()

    xg_v = xg.rearrange("(hc p) t -> p hc t", p=128)     # [128, 8, C]
    yp_v = yp.rearrange("(hc p) t -> p hc t", p=128)     # [128, 8, C]

    with tile.TileContext(nc) as tc:
        with (
            tc.tile_pool(name="const", bufs=1) as cpool,
            tc.tile_pool(name="w", bufs=2) as wpool,
            tc.tile_pool(name="inner", bufs=1) as ipool,
            tc.tile_pool(name="work", bufs=3) as spool,
            tc.tile_pool(name="psum", bufs=2, space="PSUM") as psum,
        ):
            # PE clock warm-up: the tensor engine p-state ramps over ~3us of
            # continuous execution. Burn that ramp on dummy matmuls (no DMA
            # deps, SBUF tile memset by Pool) while the first input DMAs are
            # in flight, so real matmuls run at full clock from the start.
            warm = cpool.tile([128, 512], f16, tag="warm")
            nc.vector.memset(warm[:], 0.0)
            for _ in range(6):
                wps = psum.tile([128, 512], f32, tag="up", bufs=4)
                nc.tensor.matmul(wps[:], warm[:, :128], warm[:],
                                 start=True, stop=True)

            # All up/gate weights stay resident in SBUF (8 MB), loaded once.
            wu_sb = [cpool.tile([128, HC, 128], f16, tag=f"wu{fc}",
                                name=f"wu{fc}") for fc in range(FC)]
            wg_sb = [cpool.tile([128, HC, 128], f16, tag=f"wg{fc}",
                                name=f"wg{fc}") for fc in range(FC)]
            xq = [cpool.tile([128, HC, n], f16, tag=f"xg{tn}",
                             name=f"xg{tn}")
                  for tn, (st, n) in enumerate(CHUNKS)]

            # DMA issue order is latency-critical at the start: fc0 weights,
            # then chunk-0 tokens in 2-hc pieces (so the first up-chain
            # starts after ~0.5 MB instead of the whole input), then the
            # next couple of weight tiles, then everything else. cw is only
            # needed in phase C so it loads last.
            def load_w(fc):
                nc.sync.dma_start(wu_sb[fc][:], wu[:, fc, :, :])
                nc.sync.dma_start(wg_sb[fc][:], wg[:, fc, :, :])

            # First up-chain's deps land finest-grained and first: half of
            # wu0, first xg piece, rest of wu0, next xg piece, wg0, ...
            nc.sync.dma_start(wu_sb[0][:, 0:4, :], wu[:, 0, 0:4, :])
            nc.sync.dma_start(xq[0][:, 0:2, :], xg_v[:, 0:2, 0:512])
            nc.sync.dma_start(xq[0][:, 2:4, :], xg_v[:, 2:4, 0:512])
            nc.sync.dma_start(wu_sb[0][:, 4:8, :], wu[:, 0, 4:8, :])
            nc.sync.dma_start(wg_sb[0][:], wg[:, 0, :, :])
            nc.sync.dma_start(xq[0][:, 4:6, :], xg_v[:, 4:6, 0:512])
            nc.sync.dma_start(xq[0][:, 6:8, :], xg_v[:, 6:8, 0:512])
            load_w(1)
            nc.sync.dma_start(xq[2][:], xg_v[:, :, 1024:C])
            for fc in range(2, 6):
                load_w(fc)
            for hp in range(0, HC, 2):
                nc.sync.dma_start(xq[1][:, hp:hp + 2, :],
                                  xg_v[:, hp:hp + 2, 512:1024])
            for fc in range(6, 9):
                load_w(fc)
            cw_sb = cpool.tile([128, C], f32)
            nc.sync.dma_start(cw_sb[:], cw[:])
            for fc in range(9, FC):
                load_w(fc)

            # ---- Phase A: up/gate matmuls + silu -> inner (fp16) ----
            # tn0 pass first (only needs xg chunk 0, so PE reaches steady
            # state as soon as fc0's weights and the first xg piece land),
            # with each short tn2 tile interleaved between the long
            # tn0 chains (64-wide): its Act/DVE consumers then have a 3.4us chain of
            # slack, instead of stalling the PE sequencer back-to-back.
            inner = [[None] * NT for _ in range(FC)]

            def ffn_tile(tn, fc):
                st, n = CHUNKS[tn]
                up_ps = psum.tile([128, 512], f32, tag="up", bufs=4,
                                  name="up_ps")
                for hc in range(HC):
                    nc.tensor.matmul(up_ps[:, :n], wu_sb[fc][:, hc, :],
                                     xq[tn][:, hc, :],
                                     start=(hc == 0), stop=(hc == HC - 1))
                # "y" tag shared with phase C (never live together): 4+4
                # buffers across both tags hides chain-start sem waits.
                gate_ps = psum.tile([128, 512], f32, tag="y", bufs=4,
                                    name="gate_ps")
                for hc in range(HC):
                    nc.tensor.matmul(gate_ps[:, :n], wg_sb[fc][:, hc, :],
                                     xq[tn][:, hc, :],
                                     start=(hc == 0), stop=(hc == HC - 1))
                sg_sb = spool.tile([128, 512], f32, tag="sg", name="sg_sb")
                nc.scalar.activation(sg_sb[:, :n], up_ps[:, :n],
                                     mybir.ActivationFunctionType.Sigmoid)
                silu_sb = spool.tile([128, 512], f32, tag="silu",
                                     name="silu_sb")
                nc.vector.tensor_mul(silu_sb[:, :n], sg_sb[:, :n],
                                     up_ps[:, :n])
                it = ipool.tile([128, n], f16, tag=f"i{fc}_{tn}",
                                name=f"i{fc}_{tn}")
                nc.vector.tensor_mul(it[:], silu_sb[:, :n], gate_ps[:, :n])
                inner[fc][tn] = it

            for fc in range(FC):
                ffn_tile(0, fc)
                if fc >= 1:
                    ffn_tile(2, fc - 1)
            ffn_tile(2, FC - 1)
            for fc in range(FC):
                ffn_tile(1, fc)

            # ---- Phase C: down matmul + combine scale ----
            for hc in range(HC):
                wd_t = wpool.tile([128, FC, 128], f16, tag="wd")
                nc.sync.dma_start(wd_t[:], wd[:, hc, :, :])
                for tn, (st, n) in enumerate(CHUNKS):
                    if hc == HC - 1 and tn == 1:
                        # Second-to-last chunk: run it as two 256-wide
                        # half-chains so each half's mul+store flows while
                        # the next half computes -- its DMA-completion
                        # semaphore then fires before the final chunk's,
                        # taking this store off the drain's critical path.
                        for h0, h1, eng in ((0, 256, nc.gpsimd),
                                            (256, n, nc.sync)):
                            yh_ps = psum.tile([128, 512], f32, tag="y",
                                              bufs=4, name="yh_ps")
                            for fcj in range(FC):
                                nc.tensor.matmul(
                                    yh_ps[:, :h1 - h0], wd_t[:, fcj, :],
                                    inner[fcj][tn][:, h0:h1],
                                    start=(fcj == 0), stop=(fcj == FC - 1))
                            yh_sb = spool.tile([128, 512], f32, tag="ysb",
                                               name="yh_sb")
                            nc.vector.tensor_mul(yh_sb[:, :h1 - h0],
                                                 yh_ps[:, :h1 - h0],
                                                 cw_sb[:, st + h0:st + h1])
                            eng.dma_start(yp_v[:, hc, st + h0:st + h1],
                                          yh_sb[:, :h1 - h0])
                        continue
                    y_ps = psum.tile([128, 512], f32, tag="y", bufs=4)
                    for fcj in range(FC):
                        nc.tensor.matmul(y_ps[:, :n], wd_t[:, fcj, :],
                                         inner[fcj][tn][:],
                                         start=(fcj == 0),
                                         stop=(fcj == FC - 1))
                    y_sb = spool.tile([128, 512], f32, tag="ysb")
                    nc.vector.tensor_mul(y_sb[:, :n], y_ps[:, :n],
                                         cw_sb[:, st:st + n])
                    # Alternate store queues (Pool / Act) so consecutive
                    # stores don't serialize behind one DGE; the very last
                    # store gets the Act queue to itself so the tail is
                    # just mul -> issue -> 48KB transfer -> drain.
                    if hc == HC - 1:
                        eng = nc.scalar if tn == NT - 1 else nc.gpsimd
                    else:
                        eng = (nc.gpsimd if (hc * NT + tn) % 2 == 0
                               else nc.scalar)
                    eng.dma_start(yp_v[:, hc, st:st + n], y_sb[:, :n])

    nc.compile()
    return nc


_CACHED = None


def _get_program():
    global _CACHED
    if _CACHED is None:
        _CACHED = build()
    return _CACHED


def _route(x, gw):
    """Host router: float64 logits give the true prob ordering (softmax is
    monotonic, so top-2 of probs == top-2 of logits); verified to match the
    jax f32 reference selection exactly on the reference inputs."""
    logits = x.astype(np.float64) @ gw.astype(np.float64)
    m = logits.max(axis=-1, keepdims=True)
    e = np.exp(logits - m)
    probs = e / e.sum(axis=-1, keepdims=True)
    ar = np.arange(T)
    i1 = probs.argmax(axis=-1)
    p2 = probs.copy()
    p2[ar, i1] = -np.inf
    i2 = p2.argmax(axis=-1)
    w1 = probs[ar, i1]
    w2 = probs[ar, i2]
    s = w1 + w2
    return i1, i2, (w1 / s).astype(np.float32), (w2 / s).astype(np.float32)


def _tile_ug(w):
    """[H, F] -> [128, FC, HC, 128] fp16 (p, f-tile, h-row, f)."""
    return np.ascontiguousarray(
        np.asarray(w, np.float16).reshape(HC, 128, FC, 128)
        .transpose(1, 2, 0, 3))


def _tile_d(w):
    """[F, H] -> [128, HC, FC, 128] fp16 (p, h-tile, f-row, h)."""
    return np.ascontiguousarray(
        np.asarray(w, np.float16).reshape(FC, 128, HC, 128)
        .transpose(1, 2, 0, 3))


def kernel(hidden_states, gate_w, w_up, w_gate, w_down):
    nc = _get_program()
    x = np.asarray(hidden_states, np.float32).reshape(T, H)
    gw = np.asarray(gate_w, np.float32)
    i1, i2, w1, w2 = _route(x, gw)

    # Per-expert token index lists + combine weights.
    sel = np.concatenate([i1, i2])              # [2T]
    wgt = np.concatenate([w1, w2])              # [2T]
    tok = np.concatenate([np.arange(T), np.arange(T)])
    order = np.argsort(sel, kind="stable")
    sel_s, wgt_s, tok_s = sel[order], wgt[order], tok[order]
    bounds = np.searchsorted(sel_s, np.arange(E + 1))
    idx_e = [tok_s[bounds[e]:bounds[e + 1]] for e in range(E)]
    wgt_e = [wgt_s[bounds[e]:bounds[e + 1]] for e in range(E)]

    wu16 = [_tile_ug(w_up[e]) for e in range(E)]
    wg16 = [_tile_ug(w_gate[e]) for e in range(E)]
    wd16 = [_tile_d(w_down[e]) for e in range(E)]

    y = np.zeros((T, H), np.float32)
    max_n = max(len(ix) for ix in idx_e)
    overflow = sum(max(0, len(ix) - C) for ix in idx_e)
    host_tail = 0 < overflow <= OVERFLOW_MAX
    # 1 pass normally; a handful of overflow tokens (count drift in a
    # different environment) go to the host in f32, anything larger gets
    # more device passes.
    n_pass = 1 if host_tail else max(1, -(-max_n // C))
    for p in range(n_pass):
        in_maps = []
        ns = []
        for e in range(E):
            ix = idx_e[e][p * C:(p + 1) * C]
            w = wgt_e[e][p * C:(p + 1) * C]
            n = len(ix)
            ns.append(n)
            xg = np.zeros((H, C), np.float16)
            if n:
                xg[:, :n] = x[ix].T
            cwb = np.zeros((128, C), np.float32)
            if n:
                cwb[:, :n] = w[None, :]
            in_maps.append({
                "xg": xg, "wu": wu16[e], "wg": wg16[e], "wd": wd16[e],
                "cw": cwb,
            })
        res = run_bass_kernel_spmd(nc, in_maps, list(range(N_CORES)))
        for e in range(E):
            n = ns[e]
            if n:
                ix = idx_e[e][p * C:(p + 1) * C]
                y[ix] += np.asarray(res.results[e]["yp"][:, :n].T, np.float32)

    if host_tail:
        def silu(v):
            return v / (1.0 + np.exp(-v))
        for e in range(E):
            ix = idx_e[e][C:]
            if not len(ix):
                continue
            w = wgt_e[e][C:]
            xo = x[ix]                                      # [m, H] f32
            wu_f = np.asarray(w_up[e], np.float32)
            wg_f = np.asarray(w_gate[e], np.float32)
            wd_f = np.asarray(w_down[e], np.float32)
            inner_o = silu(xo @ wu_f) * (xo @ wg_f)
            y[ix] += (inner_o @ wd_f) * w[:, None]

    return y.reshape(B, S, H)
